# revision 4
# baseline (speedup 1.0000x reference)
"""Trainium2 Bass kernel for nn_Memory_15118284882400 (scatter_memory).

Reference computation (B=16, C=512, H=64, W=64, M=512, N=B*H*W=65536):
  qr = l2norm(query, dim=C) as [N, C]; score = qr @ keys.T  [N, M]
  s_query = softmax(score, axis=0); s_memory = softmax(score, axis=1)
  top2 per row -> losses; cm = s_memory @ keys
  updated_query = concat([qr, cm]) channel-major; updated_orig = cm
  g = argmax row; coeff = s_query[i,g]/colmax[g] = exp(score[i,g] - cmax[g])
  qu = segment_sum(coeff * qr, g); updated_memory = l2norm(qu + keys)

Sharding: data-parallel over tokens, 8192 tokens (2 batches) per core; every
core holds the full keys. Since scores are cosines in [-1, 1], softmax needs
no max-subtraction: s_query = exp(score) / colsumexp with a single global
column sum; colmax is only needed for the coeff scale exp(-cmax[g]), which
factors out of the segment-sum as a per-slot scale. Both column stats are
combined across cores with a 2 KiB AllReduce mid-kernel. Segment-sum
partials, loss partials, and the final updated_memory normalization are
reduced/assembled on the host (tiny [512, 512] work).

Per-core structure (supertile = 512 tokens, 16 supertiles):
  pass 1: load qT [c, tok], l2-normalize (sum-of-squares via ones-matmul on
          PE, reciprocal broadcast via k=1 matmul), keep normalized qrT
          resident in SBUF (fp32, 16 MiB), write updated_query[:, :C]
          shard, compute scoreT in fp32r for the column stats (colsumexp
          via activation accum_out, colmax via free-dim reduce).
  mid:    AllReduce(add) colsumexp, AllReduce(max) colmax.
  pass 2: recompute score in exact fp32 [tok, m] (top-1/2 selection must be
          bit-faithful; fp32r's ~1e-5 error flips argmaxes), E = exp(score)
          with fused row sums; s_memory = E/rowsum and s_query = E*F tiles
          -> HBM; B = onehot(argmax)*E in fp32r; E_T via PE transposes for
          cm = (keys.T @ E_T) * invrowsum -> HBM (written once, used for
          both updated_query[:, C:] and updated_orig on host);
          qu += B.T @ qr accumulated in PSUM (fp32r matmuls).
  epilogue: qu *= exp(-cmax) per slot, write qu + loss partials.
"""
import sys
sys.path.insert(0, "/opt/trn_rl_repo")

import numpy as np
from contextlib import ExitStack

import concourse.bass as bass
import concourse.bacc as bacc
import concourse.tile as tile
from concourse import mybir
from concourse.bass_utils import run_bass_kernel_spmd

F32 = mybir.dt.float32
F32R = mybir.dt.float32r
AF = mybir.ActivationFunctionType
ALU = mybir.AluOpType

B, C, H, W = 16, 512, 64, 64
M = 512
NCORES = 8
BPC = B // NCORES            # batches per core = 2
TPC = BPC * H * W            # tokens per core = 8192
ST = 512                     # supertile tokens
NST = TPC // ST              # 16 supertiles
NC4 = C // 128               # 4 c-chunks
NM4 = M // 128               # 4 m-chunks
NT4 = ST // 128              # 4 tok-chunks per supertile


def build_nc():
    nc = bacc.Bacc(trn_type="TRN2", target_bir_lowering=False, debug=False,
                   num_devices=NCORES)
    q_d = nc.dram_tensor("q", [BPC, C, H * W], F32, kind="ExternalInput")
    keys_d = nc.dram_tensor("keys", [M, C], F32, kind="ExternalInput")

    uq1_d = nc.dram_tensor("uq1", [BPC, C, H * W], F32, kind="ExternalOutput")
    cm_d = nc.dram_tensor("cm", [BPC, C, H * W], F32, kind="ExternalOutput")
    smem_d = nc.dram_tensor("smem", [TPC, M], F32, kind="ExternalOutput")
    sq_d = nc.dram_tensor("sq", [TPC, M], F32, kind="ExternalOutput")
    qu_d = nc.dram_tensor("qu", [M, C], F32, kind="ExternalOutput")
    loss_d = nc.dram_tensor("loss", [128, 8], F32, kind="ExternalOutput")

    # collective bounce buffers + row-reshape scratch
    ccs_in = nc.dram_tensor("ccs_in", [128, NM4], F32)
    ccs_out = nc.dram_tensor("ccs_out", [128, NM4], F32, addr_space="Shared")
    ccm_in = nc.dram_tensor("ccm_in", [128, NM4], F32)
    ccm_out = nc.dram_tensor("ccm_out", [128, NM4], F32, addr_space="Shared")
    frow_dram = nc.dram_tensor("frow_dram", [M], F32)

    with tile.TileContext(nc) as tc, ExitStack() as ctx:
        res = ctx.enter_context(tc.tile_pool(name="res", bufs=1))
        dramp = ctx.enter_context(tc.tile_pool(name="dramp", bufs=2,
                                               space="DRAM"))

        # ---------------- prologue: constants ----------------
        ident = res.tile([128, 128], F32, tag="ident")
        nc.gpsimd.memset(ident[:], 0.0)
        nc.gpsimd.affine_select(
            out=ident[:], in_=ident[:], compare_op=ALU.not_equal,
            fill=1.0, base=0, pattern=[[-1, 128]], channel_multiplier=1)
        ones_col = res.tile([128, 1], F32, tag="ones_col")
        nc.vector.memset(ones_col[:], 1.0)
        ones_row = res.tile([1, 128], F32, tag="ones_row")
        nc.vector.memset(ones_row[:], 1.0)

        # keys: [m,c] fp32r for the cm matmul; keysT [c,m] fp32 (pass-2
        # score rhs) and fp32r (pass-1 scoreT lhsT)
        keys_mc_r = res.tile([128, NM4, C], F32R, tag="keys_mc_r")
        keysT = res.tile([128, NC4, M], F32, tag="keysT")
        keysT_r = res.tile([128, NC4, M], F32R, tag="keysT_r")

        # resident normalized qrT (fp32): [c128, cj, tok]
        qrT = res.tile([128, NC4, TPC], F32, tag="qrT")

        # stats accumulators ([m128, mj] layout)
        cse_acc = res.tile([128, NM4], F32, tag="cse_acc")
        cmx_acc = res.tile([128, NM4], F32, tag="cmx_acc")
        nc.vector.memset(cse_acc[:], 0.0)
        nc.vector.memset(cmx_acc[:], -1e30)
        comp_acc = res.tile([128, NT4], F32, tag="comp_acc")
        sep_acc = res.tile([128, NT4], F32, tag="sep_acc")
        nc.vector.memset(comp_acc[:], 0.0)
        nc.vector.memset(sep_acc[:], 0.0)
        Fb = res.tile([128, M], F32, tag="Fb")
        icmx = res.tile([128, NM4], F32, tag="icmx")

        with tc.tile_pool(name="pro", bufs=2) as pro, \
             tc.tile_pool(name="props", bufs=2, space="PSUM") as props:
            ksb = []
            for mj in range(NM4):
                kt = pro.tile([128, C], F32, tag=f"ksb{mj}", bufs=1)
                nc.sync.dma_start(kt[:], keys_d.ap()[mj * 128:(mj + 1) * 128, :])
                nc.scalar.copy(keys_mc_r[:, mj, :], kt[:])
                ksb.append(kt)
            for cj in range(NC4):
                tp = props.tile([128, M], F32, tag="ktp")
                for mj in range(NM4):
                    nc.tensor.transpose(
                        tp[:, mj * 128:(mj + 1) * 128],
                        ksb[mj][:, cj * 128:(cj + 1) * 128], ident[:])
                nc.vector.tensor_copy(keysT[:, cj, :], tp[:])
                nc.scalar.copy(keysT_r[:, cj, :], tp[:])

        # ---------------- pass 1 ----------------
        with tc.tile_pool(name="p1", bufs=1) as p1, \
             tc.tile_pool(name="p1b", bufs=2) as p1b, \
             tc.tile_pool(name="p1ps", bufs=3, space="PSUM") as p1ps:
            for st in range(NST):
                b, hw0 = st // 8, (st % 8) * ST
                t0 = st * ST
                qT = p1.tile([128, NC4, ST], F32, tag="qT")
                for cj in range(NC4):
                    nc.sync.dma_start(
                        qT[:, cj, :],
                        q_d.ap()[b, cj * 128:(cj + 1) * 128, hw0:hw0 + ST])
                qsq = p1.tile([128, NC4, ST], F32, tag="qsq")
                for cj in range(NC4):
                    nc.scalar.square(qsq[:, cj, :], qT[:, cj, :])
                ss = p1ps.tile([1, ST], F32, tag="ps")
                for cj in range(NC4):
                    nc.tensor.matmul(ss[:], ones_col[:], qsq[:, cj, :],
                                     start=(cj == 0), stop=(cj == NC4 - 1))
                invn = p1b.tile([1, ST], F32, tag="invn")
                nc.scalar.sqrt(invn[:], ss[:])
                nc.vector.tensor_scalar_max(invn[:], invn[:], 1e-12)
                nc.vector.reciprocal(invn[:], invn[:])
                invB = p1ps.tile([128, ST], F32, tag="ps")
                nc.tensor.matmul(invB[:], ones_row[:], invn[:],
                                 start=True, stop=True)
                qrT_r = p1.tile([128, NC4, ST], F32R, tag="qrT_r")
                for cj in range(NC4):
                    nc.vector.tensor_tensor(
                        out=qrT[:, cj, t0:t0 + ST], in0=qT[:, cj, :],
                        in1=invB[:], op=ALU.mult)
                    nc.sync.dma_start(
                        uq1_d.ap()[b, cj * 128:(cj + 1) * 128, hw0:hw0 + ST],
                        qrT[:, cj, t0:t0 + ST])
                    nc.scalar.copy(qrT_r[:, cj, :], qrT[:, cj, t0:t0 + ST])
                # scoreT (fp32r) -> column stats
                blk_cse = p1b.tile([128, NM4], F32, tag="blk_cse")
                blk_cmx = p1b.tile([128, NM4], F32, tag="blk_cmx")
                for mj in range(NM4):
                    stp = p1ps.tile([128, ST], F32, tag="ps")
                    for cj in range(NC4):
                        nc.tensor.matmul(
                            stp[:], keysT_r[:, cj, mj * 128:(mj + 1) * 128],
                            qrT_r[:, cj, :],
                            start=(cj == 0), stop=(cj == NC4 - 1))
                    etrash = p1b.tile([128, ST], F32, tag="etrash")
                    nc.scalar.activation(etrash[:], stp[:], AF.Exp,
                                         accum_out=blk_cse[:, mj:mj + 1])
                    nc.vector.reduce_max(blk_cmx[:, mj:mj + 1], stp[:],
                                         axis=mybir.AxisListType.X)
                nc.vector.tensor_tensor(out=cse_acc[:], in0=cse_acc[:],
                                        in1=blk_cse[:], op=ALU.add)
                nc.vector.tensor_tensor(out=cmx_acc[:], in0=cmx_acc[:],
                                        in1=blk_cmx[:], op=ALU.max)

        # ---------------- mid: AllReduce of column stats ----------------
        nc.sync.dma_start(ccs_in.ap(), cse_acc[:])
        nc.gpsimd.collective_compute(
            "AllReduce", ALU.add, replica_groups=[list(range(NCORES))],
            ins=[ccs_in.ap()], outs=[ccs_out.ap()])
        nc.sync.dma_start(ccm_in.ap(), cmx_acc[:])
        nc.gpsimd.collective_compute(
            "AllReduce", ALU.max, replica_groups=[list(range(NCORES))],
            ins=[ccm_in.ap()], outs=[ccm_out.ap()])
        with tc.tile_pool(name="mid", bufs=1) as mid, \
             tc.tile_pool(name="midps", bufs=1, space="PSUM") as midps:
            gcse = mid.tile([128, NM4], F32, tag="gcse")
            gcmx = mid.tile([128, NM4], F32, tag="gcmx")
            nc.sync.dma_start(gcse[:], ccs_out.ap())
            nc.sync.dma_start(gcmx[:], ccm_out.ap())
            finv = mid.tile([128, NM4], F32, tag="finv")
            nc.vector.reciprocal(finv[:], gcse[:])
            nc.sync.dma_start(
                frow_dram.ap().rearrange("(j p) -> p j", p=128), finv[:])
            frow = mid.tile([1, M], F32, tag="frow")
            nc.sync.dma_start(frow[:], frow_dram.ap()[None, :])
            fb_ps = midps.tile([128, M], F32, tag="fb")
            nc.tensor.matmul(fb_ps[:], ones_row[:], frow[:],
                             start=True, stop=True)
            nc.scalar.copy(Fb[:], fb_ps[:])
            nc.scalar.activation(icmx[:], gcmx[:], AF.Exp, scale=-1.0)

        # ---------------- pass 2 ----------------
        with tc.tile_pool(name="qup", bufs=1, space="PSUM") as qup, \
             tc.tile_pool(name="p2", bufs=1) as p2, \
             tc.tile_pool(name="p2b", bufs=2) as p2b, \
             tc.tile_pool(name="p2ps", bufs=3, space="PSUM") as p2ps:
            qu_ps = [qup.tile([128, C], F32, tag=f"qu{mj}", name=f"qu{mj}")
                     for mj in range(NM4)]
            for st in range(NST):
                b, hw0 = st // 8, (st % 8) * ST
                t0 = st * ST
                first, last = (st == 0), (st == NST - 1)
                rm = p2b.tile([128, NT4, 8], F32, tag="rm")
                rs = p2b.tile([128, NT4], F32, tag="rs")
                rsi = p2b.tile([128, NT4], F32, tag="rsi")
                ET = p2.tile([128, NM4, ST], F32R, tag="ET")
                Es = []
                for t in range(NT4):
                    tok = t0 + t * 128
                    sc = p2ps.tile([128, M], F32, tag="ps")
                    for cj in range(NC4):
                        nc.tensor.matmul(
                            sc[:], qrT[:, cj, tok:tok + 128],
                            keysT[:, cj, :],
                            start=(cj == 0), stop=(cj == NC4 - 1))
                    E = p2b.tile([128, M], F32, tag="E", bufs=4)
                    nc.scalar.activation(E[:], sc[:], AF.Exp,
                                         accum_out=rs[:, t:t + 1])
                    nc.vector.max(rm[:, t, :], E[:])
                    Es.append(E)
                nc.vector.reciprocal(rsi[:], rs[:])
                for t in range(NT4):
                    tok = t0 + t * 128
                    E = Es[t]
                    smem = p2b.tile([128, M], F32, tag="smem")
                    nc.scalar.mul(smem[:], E[:], rsi[:, t:t + 1])
                    nc.sync.dma_start(smem_d.ap()[tok:tok + 128, :], smem[:])
                    sq = p2b.tile([128, M], F32, tag="sq")
                    nc.vector.tensor_tensor(out=sq[:], in0=E[:], in1=Fb[:],
                                            op=ALU.mult)
                    nc.sync.dma_start(sq_d.ap()[tok:tok + 128, :], sq[:])
                    p1m = p2b.tile([128, M], F32, tag="p1m", bufs=1)
                    nc.vector.tensor_scalar(p1m[:], E[:], rm[:, t, 0:1],
                                            None, ALU.is_equal)
                    Bt = p2b.tile([128, M], F32R, tag="Bt")
                    nc.gpsimd.tensor_tensor(out=Bt[:], in0=p1m[:], in1=E[:],
                                            op=ALU.mult)
                    # E_T blocks (fp32r) for the cm matmul
                    for mj in range(NM4):
                        tp2 = p2ps.tile([128, 128], F32, tag="ps")
                        nc.tensor.transpose(
                            tp2[:], E[:, mj * 128:(mj + 1) * 128], ident[:])
                        nc.scalar.copy(ET[:, mj, t * 128:(t + 1) * 128],
                                       tp2[:])
                    # qr tok-major (fp32r) via PE transpose of resident qrT
                    qr_ps = p2ps.tile([128, C], F32, tag="ps")
                    for cj in range(NC4):
                        nc.tensor.transpose(
                            qr_ps[:, cj * 128:(cj + 1) * 128],
                            qrT[:, cj, tok:tok + 128], ident[:])
                    qr_sb = p2b.tile([128, C], F32R, tag="qr_sb")
                    nc.scalar.copy(qr_sb[:], qr_ps[:])
                    for mj in range(NM4):
                        nc.tensor.matmul(
                            qu_ps[mj][:], Bt[:, mj * 128:(mj + 1) * 128],
                            qr_sb[:],
                            start=(first and t == 0),
                            stop=(last and t == NT4 - 1),
                            skip_group_check=True)
                # rowsum-reciprocal broadcast row for the cm scale
                rrow_dram = dramp.tile([ST], F32, tag="rrow")
                nc.sync.dma_start(
                    rrow_dram[:].rearrange("(t p) -> p t", p=128), rsi[:])
                rrow = p2b.tile([1, ST], F32, tag="rrow_sb")
                nc.sync.dma_start(rrow[:], rrow_dram[:][None, :])
                rsB = p2ps.tile([128, ST], F32, tag="ps")
                nc.tensor.matmul(rsB[:], ones_row[:], rrow[:],
                                 start=True, stop=True)
                rsB_sb = p2b.tile([128, ST], F32, tag="rsB_sb")
                nc.scalar.copy(rsB_sb[:], rsB[:])
                for cj in range(NC4):
                    cmp_ = p2ps.tile([128, ST], F32, tag="ps")
                    for mj in range(NM4):
                        nc.tensor.matmul(
                            cmp_[:], keys_mc_r[:, mj, cj * 128:(cj + 1) * 128],
                            ET[:, mj, :],
                            start=(mj == 0), stop=(mj == NM4 - 1))
                    cm_sb = p2b.tile([128, ST], F32, tag="cm_sb")
                    nc.vector.tensor_tensor(out=cm_sb[:], in0=cmp_[:],
                                            in1=rsB_sb[:], op=ALU.mult)
                    nc.sync.dma_start(
                        cm_d.ap()[b, cj * 128:(cj + 1) * 128, hw0:hw0 + ST],
                        cm_sb[:])
                # loss partials from top-2 of E: rowmax = ln(Etop)
                lnr = p2b.tile([128, NT4, 2], F32, tag="lnr")
                nc.scalar.activation(lnr[:], rm[:, :, 0:2], AF.Ln)
                nc.vector.tensor_tensor(out=comp_acc[:], in0=comp_acc[:],
                                        in1=lnr[:, :, 0], op=ALU.add)
                dd = p2b.tile([128, NT4, 2], F32, tag="dd")
                nc.vector.tensor_scalar(dd[:], lnr[:], -2.0, 2.0,
                                        ALU.mult, op1=ALU.add)
                nc.vector.tensor_scalar_max(dd[:], dd[:], 0.0)
                nc.scalar.sqrt(dd[:], dd[:])
                sept = p2b.tile([128, NT4], F32, tag="sept")
                nc.vector.tensor_tensor(out=sept[:], in0=dd[:, :, 0],
                                        in1=dd[:, :, 1], op=ALU.subtract)
                nc.vector.tensor_scalar(sept[:], sept[:], 1.0, 0.0,
                                        ALU.add, op1=ALU.max)
                nc.vector.tensor_tensor(out=sep_acc[:], in0=sep_acc[:],
                                        in1=sept[:], op=ALU.add)

            # epilogue (inside pass-2 pools: reads qu_ps)
            for mj in range(NM4):
                qu_sb = p2b.tile([128, C], F32, tag="qu_sb")
                nc.scalar.mul(qu_sb[:], qu_ps[mj][:], icmx[:, mj:mj + 1])
                nc.sync.dma_start(qu_d.ap()[mj * 128:(mj + 1) * 128, :],
                                  qu_sb[:])
            lo = p2b.tile([128, 8], F32, tag="lo")
            nc.vector.tensor_copy(lo[:, 0:NT4], comp_acc[:])
            nc.vector.tensor_copy(lo[:, NT4:8], sep_acc[:])
            nc.sync.dma_start(loss_d.ap(), lo[:])

    nc.compile()
    return nc


_NC_CACHE = None


def kernel(query: np.ndarray, keys: np.ndarray):
    global _NC_CACHE
    if _NC_CACHE is None:
        _NC_CACHE = build_nc()
    nc = _NC_CACHE

    query = np.ascontiguousarray(np.asarray(query, dtype=np.float32))
    keys = np.ascontiguousarray(np.asarray(keys, dtype=np.float32))
    in_maps = []
    for i in range(NCORES):
        qs = query[i * BPC:(i + 1) * BPC].reshape(BPC, C, H * W)
        in_maps.append({"q": qs, "keys": keys})

    res = run_bass_kernel_spmd(nc, in_maps, list(range(NCORES))).results

    N = B * H * W
    updated_query = np.empty((B, 2 * C, H, W), np.float32)
    updated_orig = np.empty((B, C, H, W), np.float32)
    s_query = np.empty((N, M), np.float32)
    s_memory = np.empty((N, M), np.float32)
    qu_total = np.zeros((M, C), np.float32)
    comp_sum = 0.0
    sep_sum = 0.0
    for i in range(NCORES):
        r = res[i]
        b0 = i * BPC
        uq1 = r["uq1"].reshape(BPC, C, H, W)
        cm = r["cm"].reshape(BPC, C, H, W)
        updated_query[b0:b0 + BPC, :C] = uq1
        updated_query[b0:b0 + BPC, C:] = cm
        updated_orig[b0:b0 + BPC] = cm
        s_query[i * TPC:(i + 1) * TPC] = r["sq"]
        s_memory[i * TPC:(i + 1) * TPC] = r["smem"]
        qu_total += r["qu"]
        comp_sum += float(r["loss"][:, 0:NT4].sum(dtype=np.float64))
        sep_sum += float(r["loss"][:, NT4:8].sum(dtype=np.float64))

    um = qu_total + keys
    nrm = np.maximum(np.sqrt((um * um).sum(1, keepdims=True)), 1e-12)
    updated_memory = um / nrm
    compactness = np.float32((2.0 * N - 2.0 * comp_sum) / (N * C))
    separateness = np.float32(sep_sum / N)
    return (updated_query, updated_orig, updated_memory, s_query, s_memory,
            separateness, compactness)
